# revision 31
# baseline (speedup 1.0000x reference)
"""BiLinearAttention Trainium2 kernel (mask-compacted, host-transposed operands).

Math per batch b (one NeuronCore each, data-parallel over B=8):
    hp_proj = (hp @ W.T + b) * mp[:, None]              # (Lp, D)
    s[q, p] = hq @ hp_proj.T - 10000*(1 - mq[q]*mp[p])
    a = softmax(s, axis=q);  out[p] = sum_q a[q, p] * hq[q]

Sparsity structure (exact, not approximate):
  - Masked q rows (mq=0) receive a -10000 penalty; after the softmax max-
    subtraction exp() underflows to exactly 0.0 in fp32, so they contribute
    nothing to any output. They can be dropped from the computation.
  - Masked p rows (mp=0) have hp_proj == 0 and a uniform -10000 penalty, so
    their softmax is exactly uniform over ALL Lq positions: out[p] = mean of
    all hq rows. That vector is computed on the host.
  The device therefore only sees the COMPACTED problem: valid hq rows
  (zero-padded to LQC) x valid hp rows (zero-padded to LPC). Zero-padded q
  columns produce s == 0 <= row max, and their (tiny) exp weights multiply
  zero-padded hq rows in the output matmul, so no masking of any kind is
  needed on device.

Per-core layout strategy (compacted sizes LQC ~ LPC ~ 1100):
  - The PE contracts along the partition dim, so W, hq and hp are needed in
    transposed (contraction-major) layout. Those transposes are done on the
    HOST; the device DMAs each operand directly in its matmul layout and
    spends no PE/DVE/ACT time on input transposes.
  - Scores are computed in the transposed "sT" layout (p on partitions, q on
    the free dim) so softmax reductions run along the free dim on DVE/ACT.
  - Matmuls run in float32r (full PE rate for free size >= 256); q segments
    are chosen in {256, 384, 512} so every score matmul hits full rate.
  - Only exp(sT - max) tiles are transposed on the PE (f32r, 1.5 cyc/row)
    before the output matmul.
"""

import numpy as np
import ml_dtypes
from concourse import bacc, mybir, tile, masks
from concourse.bass_utils import run_bass_kernel_spmd

F32 = mybir.dt.float32
F32R = mybir.dt.float32r
BF16 = mybir.dt.bfloat16
EXP = mybir.ActivationFunctionType.Exp
X = mybir.AxisListType.X
MAX = mybir.AluOpType.max
MIN = mybir.AluOpType.min
ADD = mybir.AluOpType.add


def _q_segments(L):
    """Split L (multiple of 128) into segments of width 256..512 (fp32r full
    rate needs free size >= 256); a single 128 segment only if L == 128."""
    segs, off = [], 0
    rem = L
    while rem > 0:
        if rem >= 768 or rem == 512:
            w = 512
        elif rem == 640:
            w = 384
        elif rem in (256, 384):
            w = rem
        else:  # 128
            w = rem
        segs.append((off, w))
        off += w
        rem -= w
    return segs


def _p_chunks(L):
    """Split L into chunks of width 256/384 (full fp32r rate, small tiles)."""
    out, off, rem = [], 0, L
    while rem > 0:
        if rem in (256, 512):
            w = 256
        elif rem == 128:
            w = 128
        else:
            w = 384
        out.append((off, w))
        off += w
        rem -= w
    return out


def build(LQ=1152, LP=1152, D=1024, E=1024, reps=1, has_bias=False):
    """LQ/LP here are the COMPACTED sizes (both multiples of 128)."""
    nQ, nD, nE = LQ // 128, D // 128, E // 128
    nDC = D // 512                       # output d in 512-wide chunks
    chunks = _p_chunks(LP)               # p chunks for MM1 (width 256/384)
    nCH = len(chunks)
    segs = _q_segments(LQ)               # q segments for MM2/softmax
    nQS = len(segs)

    nc = bacc.Bacc("TRN2", target_bir_lowering=False, debug=False)
    hq_d = nc.dram_tensor("hq", [LQ, D], F32R, kind="ExternalInput")
    WT_d = nc.dram_tensor("WT", [E, D], F32R, kind="ExternalInput")
    hpT_d = nc.dram_tensor("hpT", [E, LP], F32R, kind="ExternalInput")
    if has_bias:
        b_d = nc.dram_tensor("b", [1, D], BF16, kind="ExternalInput")
        ones_d = nc.dram_tensor("ones_row", [1, LP], BF16, kind="ExternalInput")
    out_d = nc.dram_tensor("out", [LP, D], F32, kind="ExternalOutput")

    with tile.TileContext(nc) as tc:
        with (
            tc.tile_pool(name="big", bufs=1) as big,
            tc.tile_pool(name="hpTp", bufs=2) as hpTp,
            tc.tile_pool(name="row", bufs=2) as row,
            tc.tile_pool(name="psA", bufs=4, space="PSUM") as psA,
            tc.tile_pool(name="psT", bufs=2, space="PSUM") as psT,
            tc.tile_pool(name="psO", bufs=2, space="PSUM") as psO,
        ):
            for _rep in range(reps):
                # ---- persistent tensors ----
                hq_bf = big.tile([128, nQ, D], BF16, name="hq_bf")
                hqT = big.tile([128, nD, LQ], F32R, name="hqT")
                Wt = big.tile([128, nE, D], F32R, name="Wt", tag="Wt", bufs=2)
                hp_projT = big.tile([128, nD, 384], F32R, name="hp_projT")
                b_row = big.tile([1, D], BF16, name="b_row_sb") if has_bias else None
                ones_row = big.tile([1, LP], BF16, name="ones_sb") if has_bias else None
                ident32 = big.tile([128, 128], F32, name="ident32")
                ident = big.tile([128, 128], F32R, name="ident")
                ident_bf = big.tile([128, 128], BF16, name="ident_bf")

                masks.make_identity(nc, ident32[:])
                nc.vector.tensor_copy(ident[:], ident32[:])
                nc.vector.tensor_copy(ident_bf[:], ident32[:])
                if has_bias:
                    nc.sync.dma_start(b_row[:], b_d.ap())
                    nc.sync.dma_start(ones_row[:], ones_d.ap())

                # ---- input streams ----
                # Order matters: Wt + hpT chunk 0 gate MM1(0), whose per-et
                # accumulation chases the interleaved stream; the hq stream
                # gates the on-device hqT transposes (which fill PE idle
                # during the stream) and, via hq_bf, MM3.
                hpT_tiles = {}

                def fetch_hpT(c, wt_interleave=False):
                    p0, pw = chunks[c]
                    t = hpTp.tile([128, nE, 384], F32R, name="hpT", tag="hpT", bufs=3)
                    for et in range(nE):
                        if wt_interleave:
                            nc.sync.dma_start(Wt[:, et, :],
                                              WT_d.ap()[128 * et:128 * (et + 1), :])
                        nc.sync.dma_start(t[:, et, :pw],
                                          hpT_d.ap()[128 * et:128 * (et + 1), p0:p0 + pw])
                    hpT_tiles[c] = t

                fetch_hpT(0, wt_interleave=True)
                # hq streams through a stage ring: each tile is cast to bf16
                # (GpSimd, for the output matmul) and transposed on the PE
                # into hqT (f32r, for the score matmul).
                for qt in range(nQ):
                    q_st = hpTp.tile([128, D], F32R, name="q_st", tag="q_st", bufs=3)
                    nc.sync.dma_start(q_st[:], hq_d.ap()[128 * qt:128 * (qt + 1), :])
                    nc.gpsimd.tensor_copy(hq_bf[:, qt, :], q_st[:])
                    for g in range(nD // 4):
                        ptr = psT.tile([128, 4, 128], F32R, name="ptr", tag="ptr")
                        for j in range(4):
                            nc.tensor.matmul(ptr[:, j, :],
                                             q_st[:, 512 * g + 128 * j:512 * g + 128 * (j + 1)],
                                             ident[:], is_transpose=True, skip_group_check=True)
                        nc.vector.tensor_copy(hqT[:, 4 * g:4 * g + 4, 128 * qt:128 * (qt + 1)], ptr[:])

                def mm1(c):
                    # MM1: hp_projT[d, p_chunk] = Wt.T @ hpT (+ b rank-1 pass)
                    p0, pw = chunks[c]
                    hpT = hpT_tiles.pop(c)
                    for dt in range(nD):
                        ps1 = psA.tile([128, 512], F32, name="ps1", tag="mm12")
                        for et in range(nE):
                            nc.tensor.matmul(ps1[:, :pw], Wt[:, et, 128 * dt:128 * (dt + 1)],
                                             hpT[:, et, :pw], start=(et == 0),
                                             stop=(not has_bias and et == nE - 1))
                        if has_bias:
                            nc.tensor.matmul(ps1[:, :pw], b_row[:, 128 * dt:128 * (dt + 1)],
                                             ones_row[:, p0:p0 + pw], start=False, stop=True)
                        nc.vector.tensor_copy(hp_projT[:, dt, :pw], ps1[:, :pw])

                mm1(0)

                def row_head(c, r):
                    # MM2: sT tiles (128p x segw q), fp32 in PSUM. Flash-style
                    # softmax: per-segment local max + immediate exp (frees
                    # each PSUM bank with no cross-segment barrier), then a
                    # per-row correction exp(m_s - M) applied per segment
                    # (DVE) before the tail's transposes.
                    e_segs = []
                    neg_m = row.tile([128, nQS], F32, name="neg_m")
                    sump = row.tile([128, nQS], F32, name="sump")
                    for si, (q0, w) in enumerate(segs):
                        ps2 = psA.tile([128, 512], F32, name=f"ps2_{si}", tag="mm12")
                        for dt in range(nD):
                            nc.tensor.matmul(ps2[:, :w], hp_projT[:, dt, 128 * r:128 * (r + 1)],
                                             hqT[:, dt, q0:q0 + w],
                                             start=(dt == 0), stop=(dt == nD - 1))
                        nc.vector.tensor_reduce(neg_m[:, si:si + 1], ps2[:, :w], axis=X, op=MAX,
                                                negate=True)
                        e_seg = row.tile([128, 512], BF16, name="e_seg", bufs=2 * nQS)
                        nc.scalar.activation(e_seg[:, :w], ps2[:, :w], EXP,
                                             bias=neg_m[:, si:si + 1],
                                             accum_out=sump[:, si:si + 1])
                        e_segs.append(e_seg)
                    # row-end correction: M = max_s m_s;  c_s = exp(m_s - M)
                    neg_gmax = row.tile([128, 1], F32, name="neg_gmax")
                    nc.vector.tensor_reduce(neg_gmax[:], neg_m[:], axis=X, op=MIN)
                    c_all = row.tile([128, nQS], F32, name="c_all")
                    nc.scalar.activation(c_all[:], neg_m[:], EXP,
                                         bias=neg_gmax[:], scale=-1.0)
                    csum = row.tile([128, nQS], F32, name="csum")
                    nc.vector.tensor_mul(csum[:], c_all[:], sump[:])
                    ssum = row.tile([128, 1], F32, name="ssum")
                    nc.vector.tensor_reduce(ssum[:], csum[:], axis=X, op=ADD)
                    sinv = row.tile([128, 1], F32, name="sinv")
                    nc.vector.reciprocal(sinv[:], ssum[:])
                    for si, (q0, w) in enumerate(segs):
                        nc.vector.tensor_scalar_mul(e_segs[si][:, :w], e_segs[si][:, :w],
                                                    c_all[:, si:si + 1])
                    return (c, r, e_segs, sinv)

                def row_tail(state):
                    c, r, e_segs, sinv = state
                    p0, pw = chunks[c]
                    po0 = psO.tile([128, 512], F32, name="po0", tag="mm3")
                    po1 = psO.tile([128, 512], F32, name="po1", tag="mm3")
                    pos = [po0, po1][:nDC]
                    nseg = len(segs)
                    for si, (q0, w) in enumerate(segs):
                        e_seg = e_segs[si]
                        nj = w // 128
                        ptr = psT.tile([128, 4, 128], BF16, name="ptr_e", tag="ptr")
                        for j in range(nj):
                            nc.tensor.matmul(ptr[:, j, :], e_seg[:, 128 * j:128 * (j + 1)],
                                             ident_bf[:], is_transpose=True, skip_group_check=True)
                        et_sb = row.tile([128, 4, 128], BF16, name="et_sb", bufs=2)
                        nc.scalar.copy(et_sb[:, :nj, :], ptr[:, :nj, :])
                        for j in range(nj):
                            qt = q0 // 128 + j
                            for dc in range(nDC):
                                nc.tensor.matmul(pos[dc][:], et_sb[:, j, :],
                                                 hq_bf[:, qt, 512 * dc:512 * (dc + 1)],
                                                 start=(si == 0 and j == 0),
                                                 stop=(si == nseg - 1 and j == nj - 1))
                    out_row = row.tile([128, D], F32, name="out_row", bufs=2)
                    for dc in range(nDC):
                        nc.scalar.mul(out_row[:, 512 * dc:512 * (dc + 1)], pos[dc][:], sinv[:])
                    i = p0 // 128 + r
                    # Issued from the ACT queue: an out DMA waits on its
                    # out_row, and on the in-order SP queue that wait would
                    # head-of-line block the next rep's input stream.
                    nc.scalar.dma_start(out_d.ap()[128 * i:128 * (i + 1), :], out_row[:])

                # ---- main loop over p chunks, rows software-pipelined ----
                # Row r's tail (PE transposes + MM3) is emitted AFTER row
                # r+1's head (MM2), so the in-order PE never stalls on row
                # r's DVE/ACT softmax chain.
                pending = None
                for c in range(nCH):
                    p0, pw = chunks[c]
                    if c > 0:
                        mm1(c)
                    # prefetch next chunk's hpT so MM1(c+1) starts without a stall
                    if c + 1 < nCH:
                        fetch_hpT(c + 1)
                    for r in range(pw // 128):
                        state = row_head(c, r)
                        if pending is not None:
                            row_tail(pending)
                        pending = state
                row_tail(pending)

    nc.compile()
    return nc


_CACHE = {}


def _get_nc(shape_key):
    if shape_key not in _CACHE:
        _CACHE[shape_key] = build(*shape_key)
    return _CACHE[shape_key]


def _roundup(x, m):
    return ((x + m - 1) // m) * m


def compact_sizes(mask_hq, mask_hp):
    """Compacted device sizes (both multiples of 128, >= 256)."""
    nq = int(np.count_nonzero(np.asarray(mask_hq), axis=1).max())
    npv = int(np.count_nonzero(np.asarray(mask_hp), axis=1).max())
    LQC = max(_roundup(nq, 128), 256)
    LPC = max(_roundup(npv, 128), 256)
    return LQC, LPC


def prepare_core_inputs(hq, hp, mask_hq, mask_hp, W, b, LQC, LPC, has_bias):
    """Per-core compacted input maps (with host-side operand transposes)."""
    B, LQ, D = hq.shape
    _, LP, E = hp.shape
    WT = np.ascontiguousarray(np.asarray(W, dtype=np.float32).T)  # (E, D)
    in_maps = []
    for c in range(B):
        mq = np.asarray(mask_hq[c]) != 0
        mp = np.asarray(mask_hp[c]) != 0
        hqc = np.zeros((LQC, D), dtype=np.float32)
        hqc[: int(mq.sum())] = np.asarray(hq[c], dtype=np.float32)[mq]
        hpc = np.zeros((LPC, E), dtype=np.float32)
        hpc[: int(mp.sum())] = np.asarray(hp[c], dtype=np.float32)[mp]
        m = {
            "hq": hqc,
            "WT": WT,
            "hpT": np.ascontiguousarray(hpc.T),
        }
        if has_bias:
            m["b"] = np.ascontiguousarray(b).reshape(1, D).astype(ml_dtypes.bfloat16)
            m["ones_row"] = np.ones((1, LPC), dtype=ml_dtypes.bfloat16)
        in_maps.append(m)
    return in_maps


def kernel(hq, hp, mask_hq, mask_hp, W, b):
    B, LQ, D = hq.shape
    _, LP, E = hp.shape
    hq = np.asarray(hq, dtype=np.float32)
    hp = np.asarray(hp, dtype=np.float32)
    mask_hq = np.asarray(mask_hq)
    mask_hp = np.asarray(mask_hp)

    # Masked-p rows: softmax is exactly uniform over all LQ positions.
    mean_hq = hq.astype(np.float64).mean(axis=1).astype(np.float32)  # (B, D)

    nq_all = np.count_nonzero(mask_hq, axis=1)
    np_all = np.count_nonzero(mask_hp, axis=1)
    out = np.empty((B, LP, D), dtype=np.float32)
    if np_all.max() == 0:
        # All p masked everywhere: every output row is the uniform mean.
        return np.broadcast_to(mean_hq[:, None, :], (B, LP, D)).copy()

    # A core with zero valid q rows still attends over ALL q for its valid p
    # columns (the constant -10000 penalty cancels in softmax). Unreachable
    # for random masks; handled exactly on the host.
    fallback = [c for c in range(B) if nq_all[c] == 0]
    for c in fallback:
        out[c] = _host_core(hq[c], hp[c], mask_hq[c], mask_hp[c], W, b, mean_hq[c])
    if len(fallback) == B:
        return out

    has_bias = bool(np.any(np.asarray(b) != 0))
    LQC, LPC = compact_sizes(mask_hq, mask_hp)
    nc = _get_nc((LQC, LPC, D, E, 1, has_bias))
    in_maps = prepare_core_inputs(hq, hp, mask_hq, mask_hp, W, b, LQC, LPC, has_bias)
    res = run_bass_kernel_spmd(nc, in_maps, list(range(B)))
    for c in range(B):
        if c in fallback:
            continue
        mp = mask_hp[c] != 0
        oc = np.empty((LP, D), dtype=np.float32)
        oc[mp] = res.results[c]["out"][: int(mp.sum())]
        oc[~mp] = mean_hq[c]
        out[c] = oc
    return out


def _host_core(hq, hp, mq, mp, W, b, mean_vec):
    """Exact reference math for one batch on the host (degenerate cores)."""
    mqf = (np.asarray(mq) != 0).astype(np.float64)
    mpf = (np.asarray(mp) != 0).astype(np.float64)
    hp_proj = (hp.astype(np.float64) @ np.asarray(W, dtype=np.float64).T
               + np.asarray(b, dtype=np.float64)) * mpf[:, None]
    s = hq.astype(np.float64) @ hp_proj.T
    s = s - np.where((mqf[:, None] * mpf[None, :]) == 0, 10000.0, 0.0)
    s = s - s.max(axis=0, keepdims=True)
    e = np.exp(s)
    a = e / e.sum(axis=0, keepdims=True)
    return (a.T @ hq.astype(np.float64)).astype(np.float32)


# revision 32
# speedup vs baseline: 1.0981x; 1.0981x over previous
"""BiLinearAttention Trainium2 kernel (mask-compacted, host-transposed operands).

Math per batch b (one NeuronCore each, data-parallel over B=8):
    hp_proj = (hp @ W.T + b) * mp[:, None]              # (Lp, D)
    s[q, p] = hq @ hp_proj.T - 10000*(1 - mq[q]*mp[p])
    a = softmax(s, axis=q);  out[p] = sum_q a[q, p] * hq[q]

Sparsity structure (exact, not approximate):
  - Masked q rows (mq=0) receive a -10000 penalty; after the softmax max-
    subtraction exp() underflows to exactly 0.0 in fp32, so they contribute
    nothing to any output. They can be dropped from the computation.
  - Masked p rows (mp=0) have hp_proj == 0 and a uniform -10000 penalty, so
    their softmax is exactly uniform over ALL Lq positions: out[p] = mean of
    all hq rows. That vector is computed on the host.
  The device therefore only sees the COMPACTED problem: valid hq rows
  (zero-padded to LQC) x valid hp rows (zero-padded to LPC). Zero-padded q
  columns produce s == 0 <= row max, and their (tiny) exp weights multiply
  zero-padded hq rows in the output matmul, so no masking of any kind is
  needed on device.

Per-core layout strategy (compacted sizes LQC ~ LPC ~ 1100):
  - The PE contracts along the partition dim, so W, hq and hp are needed in
    transposed (contraction-major) layout. Those transposes are done on the
    HOST; the device DMAs each operand directly in its matmul layout and
    spends no PE/DVE/ACT time on input transposes.
  - Scores are computed in the transposed "sT" layout (p on partitions, q on
    the free dim) so softmax reductions run along the free dim on DVE/ACT.
  - Matmuls run in float32r (full PE rate for free size >= 256); q segments
    are chosen in {256, 384, 512} so every score matmul hits full rate.
  - Only exp(sT - max) tiles are transposed on the PE (f32r, 1.5 cyc/row)
    before the output matmul.
"""

import numpy as np
import ml_dtypes
from concourse import bacc, mybir, tile, masks
from concourse.bass_utils import run_bass_kernel_spmd

F32 = mybir.dt.float32
F32R = mybir.dt.float32r
BF16 = mybir.dt.bfloat16
EXP = mybir.ActivationFunctionType.Exp
X = mybir.AxisListType.X
MAX = mybir.AluOpType.max
MIN = mybir.AluOpType.min
ADD = mybir.AluOpType.add


def _q_segments(L):
    """Split L (multiple of 128) into segments of width 256..512 (fp32r full
    rate needs free size >= 256); a single 128 segment only if L == 128."""
    segs, off = [], 0
    rem = L
    while rem > 0:
        if rem >= 768 or rem == 512:
            w = 512
        elif rem == 640:
            w = 384
        elif rem in (256, 384):
            w = rem
        else:  # 128
            w = rem
        segs.append((off, w))
        off += w
        rem -= w
    return segs


def _p_chunks(L):
    """Split L into chunks of width 256/384 (full fp32r rate, small tiles)."""
    out, off, rem = [], 0, L
    while rem > 0:
        if rem in (256, 512):
            w = 256
        elif rem == 128:
            w = 128
        else:
            w = 384
        out.append((off, w))
        off += w
        rem -= w
    return out


def build(LQ=1152, LP=1152, D=1024, E=1024, reps=1, has_bias=False):
    """LQ/LP here are the COMPACTED sizes (both multiples of 128)."""
    nQ, nD, nE = LQ // 128, D // 128, E // 128
    nDC = D // 512                       # output d in 512-wide chunks
    chunks = _p_chunks(LP)               # p chunks for MM1 (width 256/384)
    nCH = len(chunks)
    segs = _q_segments(LQ)               # q segments for MM2/softmax
    nQS = len(segs)

    nc = bacc.Bacc("TRN2", target_bir_lowering=False, debug=False)
    hq_d = nc.dram_tensor("hq", [LQ, D], F32R, kind="ExternalInput")
    WT_d = nc.dram_tensor("WT", [E, D], F32R, kind="ExternalInput")
    hpT_d = nc.dram_tensor("hpT", [E, LP], F32R, kind="ExternalInput")
    if has_bias:
        b_d = nc.dram_tensor("b", [1, D], BF16, kind="ExternalInput")
        ones_d = nc.dram_tensor("ones_row", [1, LP], BF16, kind="ExternalInput")
    out_d = nc.dram_tensor("out", [LP, D], F32, kind="ExternalOutput")

    with tile.TileContext(nc) as tc:
        with (
            tc.tile_pool(name="big", bufs=1) as big,
            tc.tile_pool(name="hpTp", bufs=2) as hpTp,
            tc.tile_pool(name="row", bufs=2) as row,
            tc.tile_pool(name="psA", bufs=4, space="PSUM") as psA,
            tc.tile_pool(name="psT", bufs=2, space="PSUM") as psT,
            tc.tile_pool(name="psO", bufs=2, space="PSUM") as psO,
        ):
            for _rep in range(reps):
                # ---- persistent tensors ----
                hq_nat = big.tile([128, nQ, D], F32R, name="hq_nat")
                hq_bf = big.tile([128, nQ, D], BF16, name="hq_bf")
                hqT = big.tile([128, nD, LQ], F32R, name="hqT")
                Wt = big.tile([128, nE, D], F32R, name="Wt", tag="Wt", bufs=2)
                hp_projT = big.tile([128, nD, 384], F32R, name="hp_projT")
                b_row = big.tile([1, D], BF16, name="b_row_sb") if has_bias else None
                ones_row = big.tile([1, LP], BF16, name="ones_sb") if has_bias else None
                ident32 = big.tile([128, 128], F32, name="ident32")
                ident = big.tile([128, 128], F32R, name="ident")
                ident_bf = big.tile([128, 128], BF16, name="ident_bf")

                masks.make_identity(nc, ident32[:])
                nc.vector.tensor_copy(ident[:], ident32[:])
                nc.vector.tensor_copy(ident_bf[:], ident32[:])
                if has_bias:
                    nc.sync.dma_start(b_row[:], b_d.ap())
                    nc.sync.dma_start(ones_row[:], ones_d.ap())

                # ---- input streams ----
                # Order matters: Wt + hpT chunk 0 gate MM1(0), whose per-et
                # accumulation chases the interleaved stream; the hq stream
                # gates the on-device hqT transposes (which fill PE idle
                # during the stream) and, via hq_bf, MM3.
                hpT_tiles = {}

                def fetch_hpT(c, wt_interleave=False):
                    p0, pw = chunks[c]
                    t = hpTp.tile([128, nE, 384], F32R, name="hpT", tag="hpT", bufs=2)
                    for et in range(nE):
                        if wt_interleave:
                            nc.sync.dma_start(Wt[:, et, :],
                                              WT_d.ap()[128 * et:128 * (et + 1), :])
                        nc.sync.dma_start(t[:, et, :pw],
                                          hpT_d.ap()[128 * et:128 * (et + 1), p0:p0 + pw])
                    hpT_tiles[c] = t

                fetch_hpT(0, wt_interleave=True)
                for qt in range(nQ):
                    nc.sync.dma_start(hq_nat[:, qt, :], hq_d.ap()[128 * qt:128 * (qt + 1), :])
                # bf16 copy of hq for the (all-bf16) output matmul, cast on
                # the otherwise-idle GpSimd engine
                for qt in range(nQ):
                    nc.gpsimd.tensor_copy(hq_bf[:, qt, :], hq_nat[:, qt, :])
                # hqT from hq_nat on the PE (f32r transposes, 1.5 cyc/row);
                # runs while the W/hp/hq streams are still arriving.
                for qt in range(nQ):
                    for g in range(nD // 4):
                        ptr = psT.tile([128, 4, 128], F32R, name="ptr", tag="ptr")
                        for j in range(4):
                            nc.tensor.matmul(ptr[:, j, :],
                                             hq_nat[:, qt, 512 * g + 128 * j:512 * g + 128 * (j + 1)],
                                             ident[:], is_transpose=True, skip_group_check=True)
                        nc.vector.tensor_copy(hqT[:, 4 * g:4 * g + 4, 128 * qt:128 * (qt + 1)], ptr[:])

                def mm1(c):
                    # MM1: hp_projT[d, p_chunk] = Wt.T @ hpT (+ b rank-1 pass)
                    p0, pw = chunks[c]
                    hpT = hpT_tiles.pop(c)
                    for dt in range(nD):
                        ps1 = psA.tile([128, 512], F32, name="ps1", tag="mm12")
                        for et in range(nE):
                            nc.tensor.matmul(ps1[:, :pw], Wt[:, et, 128 * dt:128 * (dt + 1)],
                                             hpT[:, et, :pw], start=(et == 0),
                                             stop=(not has_bias and et == nE - 1))
                        if has_bias:
                            nc.tensor.matmul(ps1[:, :pw], b_row[:, 128 * dt:128 * (dt + 1)],
                                             ones_row[:, p0:p0 + pw], start=False, stop=True)
                        nc.vector.tensor_copy(hp_projT[:, dt, :pw], ps1[:, :pw])

                mm1(0)

                def row_head(c, r):
                    # MM2: sT tiles (128p x segw q), fp32 in PSUM. Flash-style
                    # softmax: per-segment local max + immediate exp (frees
                    # each PSUM bank with no cross-segment barrier), then a
                    # per-row correction exp(m_s - M) applied per segment
                    # (DVE) before the tail's transposes.
                    e_segs = []
                    neg_m = row.tile([128, nQS], F32, name="neg_m")
                    sump = row.tile([128, nQS], F32, name="sump")
                    for si, (q0, w) in enumerate(segs):
                        ps2 = psA.tile([128, 512], F32, name=f"ps2_{si}", tag="mm12")
                        for dt in range(nD):
                            nc.tensor.matmul(ps2[:, :w], hp_projT[:, dt, 128 * r:128 * (r + 1)],
                                             hqT[:, dt, q0:q0 + w],
                                             start=(dt == 0), stop=(dt == nD - 1))
                        nc.vector.tensor_reduce(neg_m[:, si:si + 1], ps2[:, :w], axis=X, op=MAX,
                                                negate=True)
                        e_seg = row.tile([128, 512], BF16, name="e_seg", bufs=max(nQS, 2))
                        nc.scalar.activation(e_seg[:, :w], ps2[:, :w], EXP,
                                             bias=neg_m[:, si:si + 1],
                                             accum_out=sump[:, si:si + 1])
                        e_segs.append(e_seg)
                    # row-end correction: M = max_s m_s;  c_s = exp(m_s - M)
                    neg_gmax = row.tile([128, 1], F32, name="neg_gmax")
                    nc.vector.tensor_reduce(neg_gmax[:], neg_m[:], axis=X, op=MIN)
                    c_all = row.tile([128, nQS], F32, name="c_all")
                    nc.scalar.activation(c_all[:], neg_m[:], EXP,
                                         bias=neg_gmax[:], scale=-1.0)
                    csum = row.tile([128, nQS], F32, name="csum")
                    nc.vector.tensor_mul(csum[:], c_all[:], sump[:])
                    ssum = row.tile([128, 1], F32, name="ssum")
                    nc.vector.tensor_reduce(ssum[:], csum[:], axis=X, op=ADD)
                    sinv = row.tile([128, 1], F32, name="sinv")
                    nc.vector.reciprocal(sinv[:], ssum[:])
                    for si, (q0, w) in enumerate(segs):
                        nc.vector.tensor_scalar_mul(e_segs[si][:, :w], e_segs[si][:, :w],
                                                    c_all[:, si:si + 1])
                    return (c, r, e_segs, sinv)

                def row_tail(state):
                    c, r, e_segs, sinv = state
                    p0, pw = chunks[c]
                    po0 = psO.tile([128, 512], F32, name="po0", tag="mm3")
                    po1 = psO.tile([128, 512], F32, name="po1", tag="mm3")
                    pos = [po0, po1][:nDC]
                    nseg = len(segs)
                    for si, (q0, w) in enumerate(segs):
                        e_seg = e_segs[si]
                        nj = w // 128
                        ptr = psT.tile([128, 4, 128], BF16, name="ptr_e", tag="ptr")
                        for j in range(nj):
                            nc.tensor.matmul(ptr[:, j, :], e_seg[:, 128 * j:128 * (j + 1)],
                                             ident_bf[:], is_transpose=True, skip_group_check=True)
                        et_sb = row.tile([128, 4, 128], BF16, name="et_sb", bufs=2)
                        nc.scalar.copy(et_sb[:, :nj, :], ptr[:, :nj, :])
                        for j in range(nj):
                            qt = q0 // 128 + j
                            for dc in range(nDC):
                                nc.tensor.matmul(pos[dc][:], et_sb[:, j, :],
                                                 hq_bf[:, qt, 512 * dc:512 * (dc + 1)],
                                                 start=(si == 0 and j == 0),
                                                 stop=(si == nseg - 1 and j == nj - 1))
                    out_row = row.tile([128, D], F32, name="out_row", bufs=2)
                    for dc in range(nDC):
                        nc.scalar.mul(out_row[:, 512 * dc:512 * (dc + 1)], pos[dc][:], sinv[:])
                    i = p0 // 128 + r
                    # Issued from the ACT queue: an out DMA waits on its
                    # out_row, and on the in-order SP queue that wait would
                    # head-of-line block the next rep's input stream.
                    nc.scalar.dma_start(out_d.ap()[128 * i:128 * (i + 1), :], out_row[:])

                # ---- main loop over p chunks ----
                for c in range(nCH):
                    p0, pw = chunks[c]
                    if c > 0:
                        mm1(c)
                    # prefetch next chunk's hpT so MM1(c+1) starts without a stall
                    if c + 1 < nCH:
                        fetch_hpT(c + 1)
                    for r in range(pw // 128):
                        row_tail(row_head(c, r))

    nc.compile()
    return nc


_CACHE = {}


def _get_nc(shape_key):
    if shape_key not in _CACHE:
        _CACHE[shape_key] = build(*shape_key)
    return _CACHE[shape_key]


def _roundup(x, m):
    return ((x + m - 1) // m) * m


def compact_sizes(mask_hq, mask_hp):
    """Compacted device sizes (both multiples of 128, >= 256)."""
    nq = int(np.count_nonzero(np.asarray(mask_hq), axis=1).max())
    npv = int(np.count_nonzero(np.asarray(mask_hp), axis=1).max())
    LQC = max(_roundup(nq, 128), 256)
    LPC = max(_roundup(npv, 128), 256)
    return LQC, LPC


def prepare_core_inputs(hq, hp, mask_hq, mask_hp, W, b, LQC, LPC, has_bias):
    """Per-core compacted input maps (with host-side operand transposes)."""
    B, LQ, D = hq.shape
    _, LP, E = hp.shape
    WT = np.ascontiguousarray(np.asarray(W, dtype=np.float32).T)  # (E, D)
    in_maps = []
    for c in range(B):
        mq = np.asarray(mask_hq[c]) != 0
        mp = np.asarray(mask_hp[c]) != 0
        hqc = np.zeros((LQC, D), dtype=np.float32)
        hqc[: int(mq.sum())] = np.asarray(hq[c], dtype=np.float32)[mq]
        hpc = np.zeros((LPC, E), dtype=np.float32)
        hpc[: int(mp.sum())] = np.asarray(hp[c], dtype=np.float32)[mp]
        m = {
            "hq": hqc,
            "WT": WT,
            "hpT": np.ascontiguousarray(hpc.T),
        }
        if has_bias:
            m["b"] = np.ascontiguousarray(b).reshape(1, D).astype(ml_dtypes.bfloat16)
            m["ones_row"] = np.ones((1, LPC), dtype=ml_dtypes.bfloat16)
        in_maps.append(m)
    return in_maps


def kernel(hq, hp, mask_hq, mask_hp, W, b):
    B, LQ, D = hq.shape
    _, LP, E = hp.shape
    hq = np.asarray(hq, dtype=np.float32)
    hp = np.asarray(hp, dtype=np.float32)
    mask_hq = np.asarray(mask_hq)
    mask_hp = np.asarray(mask_hp)

    # Masked-p rows: softmax is exactly uniform over all LQ positions.
    mean_hq = hq.astype(np.float64).mean(axis=1).astype(np.float32)  # (B, D)

    nq_all = np.count_nonzero(mask_hq, axis=1)
    np_all = np.count_nonzero(mask_hp, axis=1)
    out = np.empty((B, LP, D), dtype=np.float32)
    if np_all.max() == 0:
        # All p masked everywhere: every output row is the uniform mean.
        return np.broadcast_to(mean_hq[:, None, :], (B, LP, D)).copy()

    # A core with zero valid q rows still attends over ALL q for its valid p
    # columns (the constant -10000 penalty cancels in softmax). Unreachable
    # for random masks; handled exactly on the host.
    fallback = [c for c in range(B) if nq_all[c] == 0]
    for c in fallback:
        out[c] = _host_core(hq[c], hp[c], mask_hq[c], mask_hp[c], W, b, mean_hq[c])
    if len(fallback) == B:
        return out

    has_bias = bool(np.any(np.asarray(b) != 0))
    LQC, LPC = compact_sizes(mask_hq, mask_hp)
    nc = _get_nc((LQC, LPC, D, E, 1, has_bias))
    in_maps = prepare_core_inputs(hq, hp, mask_hq, mask_hp, W, b, LQC, LPC, has_bias)
    res = run_bass_kernel_spmd(nc, in_maps, list(range(B)))
    for c in range(B):
        if c in fallback:
            continue
        mp = mask_hp[c] != 0
        oc = np.empty((LP, D), dtype=np.float32)
        oc[mp] = res.results[c]["out"][: int(mp.sum())]
        oc[~mp] = mean_hq[c]
        out[c] = oc
    return out


def _host_core(hq, hp, mq, mp, W, b, mean_vec):
    """Exact reference math for one batch on the host (degenerate cores)."""
    mqf = (np.asarray(mq) != 0).astype(np.float64)
    mpf = (np.asarray(mp) != 0).astype(np.float64)
    hp_proj = (hp.astype(np.float64) @ np.asarray(W, dtype=np.float64).T
               + np.asarray(b, dtype=np.float64)) * mpf[:, None]
    s = hq.astype(np.float64) @ hp_proj.T
    s = s - np.where((mqf[:, None] * mpf[None, :]) == 0, 10000.0, 0.0)
    s = s - s.max(axis=0, keepdims=True)
    e = np.exp(s)
    a = e / e.sum(axis=0, keepdims=True)
    return (a.T @ hq.astype(np.float64)).astype(np.float32)
